# revision 27
# baseline (speedup 1.0000x reference)
"""MultiHeadAttention Trainium2 kernel.

Sharding: 8 cores = 2 batches x 4 head-groups (4 heads each).
Each core computes, for its batch b and heads [h0, h0+4):
  qT/kT [256, T] and v [T, 256] from xT @ w_qkv slices (channel-major),
  S^T = k q^T per head ([k, q] layout, causal folded into the mask on host),
  P = exp(S^T/sqrt(D)) * expmaskT (bf16), attention out O^T = [v|1]^T P
  (ones column gives the softmax denominators for free), O^T copied out of
  PSUM unnormalized, normalized by 1/sums (DMA-broadcast across
  partitions), then the partial output projection y_heads @ w_proj[rows].
  The host sums the 4 partial projections per batch.

Schedule: phase A (projections) is emitted in 4 n-blocks (512 tokens
each), interleaved with the attention pipeline — query-slice qs of the
attention only needs n-blocks 0..qs, so attention for qs=0 starts right
after the first n-block while n-blocks 1..3 still stream in. This keeps
the PE busy end-to-end (avoiding the K=4/8 DVFS down-clock that fires on
PE idle gaps) and hides the exp (ACT-engine) cost of the softmax under
projection matmuls. Partial output projections are injected per token
tile as soon as each query slice is normalized. All matmul operands are
bf16 (fp32 PSUM accumulate). The causal block-diagonal is trimmed
(shorter S/PV streams + smaller exp/mul) at 128-key granularity.
"""

import sys

sys.path.insert(0, "/opt/trn_rl_repo")

import ml_dtypes
import numpy as np

import concourse.bass as bass
import concourse.mybir as mybir
import concourse.tile as tile
from concourse import bacc
from concourse.bass_utils import run_bass_kernel_spmd

B, T, C, H, D = 2, 2048, 1024, 16, 64
HPC = 4  # heads per core
NCORES = 8
KC = C // 128  # 8 contraction chunks for the projections
NT = T // 128  # 16 token tiles
NQ = T // 512  # 4 query slices
F32, BF16 = mybir.dt.float32, mybir.dt.bfloat16
AF = mybir.ActivationFunctionType
NEG = np.float32(-1.0e30)

# group list (qs, g): for query-slice qs, key-group g covers k-chunks
# [2g, 2g+2) of 128 keys each; causality keeps g < 2qs+2.
GROUPS = [(qs, g) for qs in range(NQ) for g in range(2 * qs + 2)]
GIDX = {qg: i for i, qg in enumerate(GROUPS)}
QS_FIRST = {qs: GIDX[(qs, 0)] for qs in range(NQ)}

_cache = {}


def _build():
    nc = bacc.Bacc("TRN2", target_bir_lowering=False, debug=False, num_devices=NCORES)
    xt_d = nc.dram_tensor("xt", [C, T], BF16, kind="ExternalInput")
    maskt_d = nc.dram_tensor("maskt", [T, T], BF16, kind="ExternalInput")
    wqkv_d = nc.dram_tensor("wqkv", [C, 3 * HPC * D], BF16, kind="ExternalInput")
    wproj_d = nc.dram_tensor("wproj", [HPC * D, C], BF16, kind="ExternalInput")
    out_d = nc.dram_tensor("out", [T, C], BF16, kind="ExternalOutput")

    with tile.TileContext(nc) as tc:
        with (
            tc.tile_pool(name="ps", bufs=1, space="PSUM") as ps,
            tc.tile_pool(name="apool", bufs=1) as apool,
            tc.tile_pool(name="mpool", bufs=1) as mpool,
            tc.tile_pool(name="espool", bufs=8) as espool,
            tc.tile_pool(name="xpool", bufs=8) as xpool,
            tc.tile_pool(name="spool", bufs=2) as spool,
            tc.tile_pool(name="stpool", bufs=4) as stpool,
            tc.tile_pool(name="wpool", bufs=1) as wpool,
            tc.tile_pool(name="dpool", bufs=2, space="DRAM") as dpool,
        ):
            xt_sb = wpool.tile([128, KC, T], BF16, tag="xt")
            wqkv_sb = wpool.tile([128, KC, 3 * HPC * D], BF16, tag="wqkv")
            wproj_sb = wpool.tile([128, 2, C], BF16, tag="wproj")
            mask_sb = mpool.tile([128, len(GROUPS), 2, 512], BF16, tag="mask")
            xt_v = xt_d.ap().rearrange("(c p) t -> p c t", p=128)
            wqkv_v = wqkv_d.ap().rearrange("(c p) n -> p c n", p=128)
            maskt_v = maskt_d.ap().rearrange("(c p) q -> p c q", p=128)  # [128,16,T]

            # ---- input DMA issue order. Descriptor issue costs ~630ns per
            # dma_start per engine queue, so issue is split across the sync
            # and gpsimd queues, ordered to match phase-A kc-major
            # consumption (per kc: wqkv qk-cols + xt chunk + wqkv v-cols).
            def dma_mask(qs):
                g0 = QS_FIRST[qs]
                ng = 2 * qs + 2
                for a in range(0, ng, 2):
                    b = min(a + 2, ng)
                    nc.sync.dma_start(
                        out=mask_sb[:, g0 + a : g0 + b, :, :],
                        in_=maskt_v[:, 2 * a : 2 * b, qs * 512 : (qs + 1) * 512],
                    )

            for kc in range(KC):
                splits = ((0, 256), (256, 512)) if kc == 0 else ((0, 512),)
                for c0, c1 in splits:
                    nc.sync.dma_start(
                        out=wqkv_sb[:, kc, c0:c1], in_=wqkv_v[:, kc, c0:c1]
                    )
                    nc.gpsimd.dma_start(
                        out=xt_sb[:, kc, c0:c1], in_=xt_v[:, kc, c0:c1]
                    )
                nc.gpsimd.dma_start(
                    out=wqkv_sb[:, kc, 512:768], in_=wqkv_v[:, kc, 512:768]
                )
            dma_mask(0)
            for kc in range(KC):
                nc.sync.dma_start(
                    out=xt_sb[:, kc, 512:1024], in_=xt_v[:, kc, 512:1024]
                )
            dma_mask(1)
            for kc in range(KC):
                nc.gpsimd.dma_start(
                    out=xt_sb[:, kc, 1024:1536], in_=xt_v[:, kc, 1024:1536]
                )
            dma_mask(2)
            for kc in range(KC):
                nc.sync.dma_start(
                    out=xt_sb[:, kc, 1536:2048], in_=xt_v[:, kc, 1536:2048]
                )
            dma_mask(3)
            nc.sync.dma_start(
                out=wproj_sb, in_=wproj_d.ap().rearrange("(m p) n -> p m n", p=128)
            )

            # ---- persistent SBUF tiles
            qt_tiles = [
                apool.tile([128, T], BF16, tag=f"qt{m}", name=f"qt{m}") for m in range(2)
            ]
            kt_tiles = [
                apool.tile([128, T], BF16, tag=f"kt{m}", name=f"kt{m}") for m in range(2)
            ]
            v_sb = apool.tile([128, NT, HPC * 65], BF16, tag="v")
            v_4d = v_sb.rearrange("p t (h e) -> p t h e", h=HPC)
            yt_tiles = [
                apool.tile([128, T], BF16, tag=f"yt{m}", name=f"yt{m}") for m in range(2)
            ]
            ones_t = spool.tile([128, NT * HPC], F32, tag="ones", bufs=1, name="ones_t")
            nc.vector.memset(ones_t, 1.0)
            nc.scalar.activation(
                v_4d[:, :, :, 64:65],
                ones_t.rearrange("p (t h one) -> p t h one", t=NT, one=1),
                AF.Copy,
            )

            # ---- phase A block for one n-slice: qt/kt columns + v token
            # tiles [4n, 4n+4). kc-major so compute tracks DMA arrival of
            # xt/wqkv chunk kc. psum tag "sp" rotation (shared with phase B).
            def emit_ablock(n, qk_only=False):
                sl = slice(n * 512, (n + 1) * 512)
                qk_ps = ps.tile([128, 2, 512], F32, tag="sp", bufs=2, name="qk_ps")
                kt_ps = ps.tile([128, 2, 512], F32, tag="sp", bufs=2, name="kt_ps")
                if not qk_only:
                    # one psum BANK per v chain: interleaved open accumulation
                    # chains must not share a bank (two chains at 256-offsets
                    # inside one bank corrupt each other).
                    v_ps = ps.tile([128, 4, 512], F32, tag="pv", bufs=1, name="v_ps")
                for kc in range(KC):
                    for m in range(2):
                        nc.tensor.matmul(
                            qk_ps[:, m, :],
                            lhsT=wqkv_sb[:, kc, m * 128 : (m + 1) * 128],
                            rhs=xt_sb[:, kc, sl],
                            start=(kc == 0),
                            stop=(kc == KC - 1),
                        )
                    for m in range(2):
                        nc.tensor.matmul(
                            kt_ps[:, m, :],
                            lhsT=wqkv_sb[
                                :, kc, HPC * D + m * 128 : HPC * D + (m + 1) * 128
                            ],
                            rhs=xt_sb[:, kc, sl],
                            start=(kc == 0),
                            stop=(kc == KC - 1),
                        )
                    if not qk_only:
                        for i in range(4):
                            tt = 4 * n + i
                            nc.tensor.matmul(
                                v_ps[:, i, 0:256],
                                lhsT=xt_sb[:, kc, tt * 128 : (tt + 1) * 128],
                                rhs=wqkv_sb[:, kc, 2 * HPC * D : 3 * HPC * D],
                                start=(kc == 0),
                                stop=(kc == KC - 1),
                            )
                for m in range(2):
                    nc.vector.tensor_copy(qt_tiles[m][:, sl], qk_ps[:, m, :])
                for m in range(2):
                    nc.vector.tensor_copy(kt_tiles[m][:, sl], kt_ps[:, m, :])
                if not qk_only:
                    for i in range(4):
                        tt = 4 * n + i
                        nc.vector.tensor_copy(
                            v_4d[:, tt, :, 0:64],
                            v_ps[:, i, 0:256].rearrange("p (h d) -> p h d", h=HPC),
                        )

            # ---- attention pieces
            def emit_s_group(qs, g):
                """S^T matmuls for group g (k-chunks 2g, 2g+1) of q-slice qs.
                On the causal diagonal the streams are trimmed: k-chunk c
                (relative to the diag) only serves queries q >= 128c."""
                sps = []
                for h in range(HPC):
                    mh, ph = divmod(h, 2)
                    p0 = ph * 64
                    sp = ps.tile([128, 2, 512], F32, tag="sp", bufs=2, name="sp")
                    for i in range(2):
                        kc = 2 * g + i
                        c = kc - 4 * qs  # chunk position relative to diagonal
                        qoff = 128 * c if c > 0 else 0
                        nc.tensor.matmul(
                            sp[:, i, qoff:512],
                            lhsT=kt_tiles[mh][p0 : p0 + 64, kc * 128 : (kc + 1) * 128],
                            rhs=qt_tiles[mh][p0 : p0 + 64, qs * 512 + qoff : (qs + 1) * 512],
                            start=True,
                            stop=True,
                        )
                    sps.append(sp)
                return sps

            def emit_em(qs, g, sps):
                """P = exp(S) * expmask (bf16). For the second diagonal group
                (c=2,3) only the live region is processed; PV reads match."""
                gi = GIDX[(qs, g)]
                diag2 = g == 2 * qs + 1
                tiles = []
                for h in range(HPC):
                    exps = xpool.tile([128, 2, 512], BF16, tag="exps", name="exps")
                    es = espool.tile([128, 2, 512], BF16, tag="es", name="es")
                    if diag2:
                        nc.scalar.activation(
                            exps[:, :, 384:512], sps[h][:, :, 384:512], AF.Exp
                        )
                        nc.scalar.activation(
                            exps[:, 0, 256:384], sps[h][:, 0, 256:384], AF.Exp
                        )
                        nc.vector.tensor_mul(
                            es[:, :, 384:512],
                            exps[:, :, 384:512],
                            mask_sb[:, gi, :, 384:512],
                        )
                        nc.vector.tensor_mul(
                            es[:, 0, 256:384],
                            exps[:, 0, 256:384],
                            mask_sb[:, gi, 0, 256:384],
                        )
                    else:
                        nc.scalar.activation(exps, sps[h], AF.Exp)
                        nc.vector.tensor_mul(es, exps, mask_sb[:, gi, :, :])
                    tiles.append(es)
                return tiles

            pv_tiles = {}

            def emit_pv(qs, g, tiles):
                if g == 0:
                    pv_tiles[qs] = ps.tile(
                        [65, HPC, 512], F32, tag="pv", bufs=1, name="pv_all"
                    )
                nkc = 4 * qs + 4
                for h in range(HPC):
                    for i in range(2):
                        kc = 2 * g + i
                        c = kc - 4 * qs
                        qoff = 128 * c if c > 0 else 0
                        nc.tensor.matmul(
                            pv_tiles[qs][:, h, qoff:512],
                            lhsT=v_sb[:, kc, h * 65 : (h + 1) * 65],
                            rhs=tiles[h][:, i, qoff:512],
                            start=(kc == 0),
                            stop=(kc == nkc - 1),
                            skip_group_check=(qoff > 0),
                        )

            def emit_pvout(qs):
                """copy O^T (+ sums row) out of PSUM so the pv bank frees up
                for the next q-slice, then kick off the 1/sums pipeline:
                sums [1,2048] -> DRAM -> [128,16] spread -> reciprocal (bf16)
                -> DRAM -> [64,HPC,512] broadcast."""
                pvo = spool.tile([65, HPC, 512], BF16, tag="pvo", name="pvo")
                nc.vector.tensor_copy(pvo, pv_tiles.pop(qs))
                spread = spool.tile([128, 16], BF16, tag="spread", name="spread")
                nc.gpsimd.dma_start(out=spread, in_=pvo[64:65, :, :])
                rspread = spool.tile([128, 16], BF16, tag="rspread", name="rspread")
                with nc.allow_low_precision("bf16 1/sums; rel-err budget is 2e-2"):
                    nc.vector.reciprocal(rspread, spread)
                d2 = dpool.tile([HPC * 512], BF16, tag="d2", name="d2")
                nc.gpsimd.dma_start(out=d2.rearrange("(p e) -> p e", p=128), in_=rspread)
                bcast = spool.tile([64, HPC, 512], BF16, tag="bcast", name="bcast")
                bsrc = bass.AP(
                    tensor=d2.tensor,
                    offset=d2.offset,
                    ap=[[0, 64], [512, HPC], [1, 512]],
                )
                nc.gpsimd.dma_start(out=bcast, in_=bsrc)
                return pvo, bcast

            def emit_norm(qs, pvo, bcast):
                for h in range(HPC):
                    mh, ph = divmod(h, 2)
                    nc.vector.tensor_mul(
                        yt_tiles[mh][ph * 64 : (ph + 1) * 64, qs * 512 : (qs + 1) * 512],
                        pvo[0:64, h, :],
                        bcast[:, h, :],
                    )

            def emit_proj_tt(tt, eng):
                """partial projection for one token tile; staging cast on the
                given engine to balance ACT/DVE load."""
                st = stpool.tile([128, C], BF16, tag="stage", name="st")
                for ns in range(2):
                    pj_ps = ps.tile([128, 2, 512], F32, tag="sp", bufs=2, name="pj_ps")
                    for m in range(2):
                        nc.tensor.matmul(
                            pj_ps[:, 0, :],
                            lhsT=yt_tiles[m][:, tt * 128 : (tt + 1) * 128],
                            rhs=wproj_sb[:, m, ns * 512 : (ns + 1) * 512],
                            start=(m == 0),
                            stop=(m == 1),
                        )
                    nc.vector.tensor_copy(
                        st[:, ns * 512 : (ns + 1) * 512], pj_ps[:, 0, :]
                    )
                nsplit = 8 if tt == NT - 1 else (4 if tt == NT - 2 else 2)
                step = C // nsplit
                for sp0 in range(0, C, step):
                    nc.sync.dma_start(
                        out=out_d.ap()[tt * 128 : (tt + 1) * 128, sp0 : sp0 + step],
                        in_=st[:, sp0 : sp0 + step],
                    )

            # ---- interleaved schedule.
            # Pipeline over groups: step i emits S(i+2), em(i+1), PV(i).
            # A-blocks injected at fixed steps (dependencies: S of qs needs
            # n-blocks <= qs). norm after PV(qs) completes; proj token tiles
            # spread over subsequent steps.
            n = len(GROUPS)
            em_out = {}

            def stage_s(i):
                if i < n:
                    em_out[i] = (GROUPS[i], emit_s_group(*GROUPS[i]))

            def stage_em(i):
                if 0 <= i < n:
                    (qs, g), sps = em_out[i]
                    em_out[i] = ((qs, g), emit_em(qs, g, sps))

            def stage_pv(i):
                if not (0 <= i < n):
                    return None
                (qs, g), tiles = em_out.pop(i)
                emit_pv(qs, g, tiles)
                if g == 2 * qs + 1:
                    return qs
                return None

            emit_ablock(0)
            stage_s(0)
            stage_s(1)
            stage_em(0)
            emit_ablock(1)
            # A-blocks inject right after the pvout that frees their psum
            # (steps 1 and 5 are where qs0/qs1 finish). Two proj token tiles
            # per q-slice run promptly; two are deferred to the tail (steps
            # >= n) to keep the PE fed while the last norm chain drains.
            ablock_at = {1: 2, 5: 3}
            norm_q, proj_q = [], []
            tail_slot = [18]
            eng_tgl = [0]
            for i in range(n + 10):
                while norm_q and norm_q[0][0] <= i:
                    _, dq, pvo, bcast = norm_q.pop(0)
                    emit_norm(dq, pvo, bcast)
                    if dq < NQ - 1:
                        proj_q.append((i + 1, 4 * dq))
                        proj_q.append((i + 2, 4 * dq + 1))
                        for k in (2, 3):
                            proj_q.append((tail_slot[0], 4 * dq + k))
                            tail_slot[0] += 1
                    else:
                        for k in range(4):
                            proj_q.append((24 + k, 4 * dq + k))
                    proj_q.sort()
                while proj_q and proj_q[0][0] <= i:
                    _, tt = proj_q.pop(0)
                    emit_proj_tt(tt, eng_tgl[0])
                    eng_tgl[0] ^= 1
                fin = stage_pv(i)
                if fin is not None:
                    pvo, bcast = emit_pvout(fin)
                    norm_q.append((i + 1, fin, pvo, bcast))
                if i in ablock_at:
                    emit_ablock(ablock_at[i])
                stage_s(i + 2)
                stage_em(i + 1)
            while norm_q:
                _, dq, pvo, bcast = norm_q.pop(0)
                emit_norm(dq, pvo, bcast)
                for k in range(4):
                    proj_q.append((0, 4 * dq + k))
            while proj_q:
                _, tt = proj_q.pop(0)
                emit_proj_tt(tt, eng_tgl[0])
                eng_tgl[0] ^= 1

    nc.compile()
    return nc


def _get_program():
    if "nc" not in _cache:
        _cache["nc"] = _build()
    return _cache["nc"]


def _prep_in_maps(x, mask, w_qkv, w_proj, head_mask):
    x = np.asarray(x, dtype=np.float32)
    mask = np.asarray(mask, dtype=np.float32)
    w_qkv = np.asarray(w_qkv, dtype=np.float32)
    w_proj = np.asarray(w_proj, dtype=np.float32)
    head_mask = np.asarray(head_mask, dtype=np.float32)

    idx = np.arange(T)
    causal_pen = np.where(idx[:, None] > idx[None, :], NEG, np.float32(0.0))  # [k, q]

    xts, maskts = [], []
    for b in range(B):
        xts.append(np.ascontiguousarray(x[b].T).astype(ml_dtypes.bfloat16))
        em = np.exp(np.ascontiguousarray(mask[b, 0].T) + causal_pen)
        maskts.append(em.astype(ml_dtypes.bfloat16))

    in_maps = []
    for core in range(NCORES):
        b, hg = divmod(core, NCORES // B)
        h0 = hg * HPC
        wq = w_qkv[:, h0 * D : (h0 + HPC) * D] * np.float32(0.125)  # 1/sqrt(D)
        wk = w_qkv[:, C + h0 * D : C + (h0 + HPC) * D]
        wv = w_qkv[:, 2 * C + h0 * D : 2 * C + (h0 + HPC) * D]
        wqkv_c = np.ascontiguousarray(np.concatenate([wq, wk, wv], axis=1)).astype(
            ml_dtypes.bfloat16
        )
        wp = w_proj[h0 * D : (h0 + HPC) * D, :] * np.repeat(head_mask[h0 : h0 + HPC], D)[:, None]
        in_maps.append(
            {
                "xt": xts[b],
                "maskt": maskts[b],
                "wqkv": wqkv_c,
                "wproj": np.ascontiguousarray(wp.astype(ml_dtypes.bfloat16)),
            }
        )
    return in_maps


def run(inputs, trace=False, trace_cores=None):
    nc = _get_program()
    in_maps = _prep_in_maps(**inputs)
    res = run_bass_kernel_spmd(
        nc,
        in_maps,
        list(range(NCORES)),
        trace=trace,
        trace_cores=trace_cores,
    )
    out = np.zeros((B, T, C), dtype=np.float32)
    for core in range(NCORES):
        out[core // (NCORES // B)] += np.asarray(
            res.results[core]["out"], dtype=np.float32
        )
    return out, res


def kernel(x, mask, w_qkv, w_proj, head_mask):
    out, _ = run(dict(x=x, mask=mask, w_qkv=w_qkv, w_proj=w_proj, head_mask=head_mask))
    return out
